# revision 16
# baseline (speedup 1.0000x reference)
"""Segment-mean pooling (segment_sum / counts) + Linear, on 8 TRN2 NeuronCores.

Strategy: segment-ownership sharding with rank-slice packing and fp8
DoubleRow matmuls.

The host sorts segments by count and deals them round-robin across the 8
cores (so per-core load is balanced).  Within a core its 512 segments are
kept count-sorted and split into 4 PSUM blocks of 128.  The host packs
the rows of x so that chunk c of block b holds, byte-interleaved in SBUF
partition p, rows 2c and 2c+1 of block b's p-th segment (zeros where the
segment has fewer rows).  Segment-summing a chunk is then ONE fp8
DoubleRow TensorE matmul (256 rows per ~109 ns) with a constant doubled
identity as the stationary operand: psum[p, :] += row2c[p, :] +
row2c1[p, :].  Full 128-wide matmuls keep the PE clock warm (HAM) and
need no per-row index handling, no one-hots, and no overflow pass.

x is shipped as fp8 e4m3 with per-segment error-feedback quantization:
rows of one segment are quantized in sequence, carrying the running
quantization residual into the next row, so the device-side segment sum
telescopes to full-precision accuracy minus ONE element's rounding
(final rel err ~5e-3 vs the 2e-2 gate) at half the f16 DMA bytes.  The
kernel is DMA-bound at ~350 GB/s; x streams in ~0.4 MB groups whose
completions are tracked with one cumulative semaphore (per-queue FIFO).

Epilogue per block: cast sums to f16, PE-transpose to [h, s], Linear via
2 accumulated matmuls against W.T, scale rows by 1/(count+eps) with
fused bias-add, one f16 output DMA (host upcasts and unpermutes).
"""

import numpy as np
import ml_dtypes

import concourse.bass as bass
import concourse.mybir as mybir
from concourse.bass_utils import run_bass_kernel_spmd

N_CORES = 8
S_TOTAL = 4096
S_PER = S_TOTAL // N_CORES  # 512 segments per core
NBLK = 4  # PSUM blocks of 128 segments per core
H = 256
EPS = np.float32(1e-8)
F8 = ml_dtypes.float8_e4m3  # matches mybir.dt.float8e4

N_WARM = 34  # PE warmup matmuls: one uninterrupted ~3.6 us run so the
# HAM clock transition to 2.4 GHz is guaranteed before data matmuls

_graph_cache: dict = {}


def _groups(nb2):
    """x DMA groups: few large transfers (each extra DMA on the queue
    costs ~0.8 us of boundary stall, and small transfers lose HBM rate);
    split the final block so the PE trail after the last byte is short.

    Returns a list of (start_chunk, n_chunks) in global chunk indexing.
    """
    sizes = list(nb2[:-1])
    last = nb2[-1]
    tail = min(5, last - 1) if last > 1 else 0
    if tail:
        sizes.extend([last - tail, tail])
    else:
        sizes.append(last)
    gs = []
    base = 0
    for n in sizes:
        gs.append((base, n))
        base += n
    return gs


def _build(nb2: tuple) -> "bass.Bass":
    """nb2[b] = DoubleRow chunk count of block b (same on every core)."""
    f8 = mybir.dt.float8e4
    f16 = mybir.dt.float16
    f32 = mybir.dt.float32
    ktot = sum(nb2)
    off = [sum(nb2[:b]) for b in range(NBLK)]
    groups = _groups(nb2)
    # group index that completes chunk c, for wait thresholds
    g_of_chunk = {}
    for gi, (c0, n) in enumerate(groups):
        for c in range(c0, c0 + n):
            g_of_chunk[c] = gi

    nc = bass.Bass()

    xp_d = nc.declare_dram_parameter("xp", [128, ktot, 2 * H], f8, isOutput=False)
    did_d = nc.declare_dram_parameter("did", [128, 2, 128], f8, isOutput=False)
    idh_d = nc.declare_dram_parameter("idh", [128, 128], f16, isOutput=False)
    wt_d = nc.declare_dram_parameter("wt", [H, H], f16, isOutput=False)
    invc_d = nc.declare_dram_parameter("invc", [128, NBLK], f32, isOutput=False)
    bb_d = nc.declare_dram_parameter("bb", [128, H], f32, isOutput=False)
    # [partition, block, H]: contiguous DMA; the host un-interleaves
    out_d = nc.declare_dram_parameter("out", [128, NBLK, H], f16, isOutput=True)

    from contextlib import ExitStack

    with ExitStack() as ctx:
        xbb = ctx.enter_context(nc.sbuf_tensor("xbb", [128, ktot, 2 * H], f8))
        did_sb = ctx.enter_context(nc.sbuf_tensor("did_sb", [128, 2, 128], f8))
        idh_sb = ctx.enter_context(nc.sbuf_tensor("idh_sb", [128, 128], f16))
        wt_sb = ctx.enter_context(nc.sbuf_tensor("wt_sb", [128, 2, H], f16))
        invc_sb = ctx.enter_context(nc.sbuf_tensor("invc_sb", [128, NBLK], f32))
        bb_sb = ctx.enter_context(nc.sbuf_tensor("bb_sb", [128, H], f32))
        pool_sb = ctx.enter_context(nc.sbuf_tensor("pool_sb", [128, NBLK, H], f16))
        sums2_sb = ctx.enter_context(
            nc.sbuf_tensor("sums2_sb", [128, 2, S_PER], f16)
        )
        out_sb = ctx.enter_context(nc.sbuf_tensor("out_sb", [128, NBLK, H], f16))
        # every PSUM tensor padded to one full private 2 KiB bank
        ps = [
            ctx.enter_context(nc.psum_tensor(f"ps{b}", [128, 512], f32))
            for b in range(NBLK)
        ]
        ps_t = [
            ctx.enter_context(nc.psum_tensor(f"ps_t{i}", [128, 1024], f16))
            for i in range(2)
        ]
        ps_x = ctx.enter_context(nc.psum_tensor("ps_x", [128, 512], f32))
        dma_sem = ctx.enter_context(nc.semaphore("dma_sem"))
        csem = {
            name: ctx.enter_context(nc.semaphore(f"csem_{name}"))
            for name in ("did", "idh", "wt", "invc", "bb")
        }
        xsem = [
            ctx.enter_context(nc.semaphore(f"xsem{g}"))
            for g in range(len(groups))
        ]
        mmf_sem = ctx.enter_context(nc.semaphore("mmf_sem"))
        fold_sem = ctx.enter_context(nc.semaphore("fold_sem"))
        tr_sem = ctx.enter_context(nc.semaphore("tr_sem"))
        cp2_sem = ctx.enter_context(nc.semaphore("cp2_sem"))
        lin_sem = ctx.enter_context(nc.semaphore("lin_sem"))
        oe_sem = ctx.enter_context(nc.semaphore("oe_sem"))
        block = ctx.enter_context(nc.Block())

        @block.sync
        def _(sync):
            # late-needed consts only: this queue can starve for a few us
            # behind the x stream, which is fine here
            sync.dma_start(
                out=wt_sb[:, :, :],
                in_=wt_d[:, :].rearrange("(t p) j -> p t j", p=128),
            ).then_inc(csem["wt"], 16)
            sync.dma_start(out=invc_sb[:, :], in_=invc_d[:, :]).then_inc(
                csem["invc"], 16
            )
            sync.dma_start(out=bb_sb[:, :], in_=bb_d[:, :]).then_inc(
                csem["bb"], 16
            )
            sync.wait_ge(oe_sem, NBLK)
            sync.dma_start(
                out=out_d[:, :, :], in_=out_sb[:, :, :]
            ).then_inc(dma_sem, 16)
            for name in ("wt", "invc", "bb"):
                sync.wait_ge(csem[name], 16)
            sync.wait_ge(dma_sem, 16)

        @block.scalar
        def _(scalar):
            # time-critical consts FIRST on the same queue as x (FIFO):
            # idh gates the PE warmup, did gates the data matmuls
            scalar.dma_start(out=idh_sb[:, :], in_=idh_d[:, :]).then_inc(
                csem["idh"], 16
            )
            scalar.dma_start(out=did_sb[:, :, :], in_=did_d[:, :, :]).then_inc(
                csem["did"], 16
            )
            # one semaphore per group: groups can land out of order (DMAs
            # round-robin across queues), so a shared cumulative
            # threshold can't tell WHICH group completed
            for gi, (c0, n) in enumerate(groups):
                scalar.dma_start(
                    out=xbb[:, c0 : c0 + n, :],
                    in_=xp_d[:, c0 : c0 + n, :],
                ).then_inc(xsem[gi], 16)
            scalar.wait_ge(csem["idh"], 16)
            scalar.wait_ge(csem["did"], 16)
            for gi in range(len(groups)):
                scalar.wait_ge(xsem[gi], 16)

        @block.vector
        def _(vector):
            for b in range(NBLK):
                vector.wait_ge(mmf_sem, b + 1)
                vector.tensor_copy(
                    out=pool_sb[:, b, :], in_=ps[b][:, 0:H]
                ).then_inc(fold_sem, 1)
            for b in range(NBLK):
                vector.wait_ge(tr_sem, b + 1)
                vector.tensor_copy(
                    out=sums2_sb[:, 0, 128 * b : 128 * (b + 1)],
                    in_=ps_t[b % 2][:, 0:128],
                )
                vector.tensor_copy(
                    out=sums2_sb[:, 1, 128 * b : 128 * (b + 1)],
                    in_=ps_t[b % 2][:, 128:256],
                ).then_inc(cp2_sem, 1)
            vector.wait_ge(csem["invc"], 16)
            vector.wait_ge(csem["bb"], 16)
            for b in range(NBLK):
                vector.wait_ge(lin_sem, b + 1)
                vector.scalar_tensor_tensor(
                    out=out_sb[:, b, :],
                    in0=ps[b][:, 0:H],
                    scalar=invc_sb[:, b : b + 1],
                    in1=bb_sb[:, :],
                    op0=mybir.AluOpType.mult,
                    op1=mybir.AluOpType.add,
                ).then_inc(oe_sem, 1)

        @block.tensor
        def _(tensor):
            # HAM warmup: sustained activity from the earliest moment so
            # the PE is at 2.4 GHz when the first data group lands
            tensor.wait_ge(csem["idh"], 16)
            for _ in range(N_WARM):
                tensor.matmul(
                    ps_x[:, 0:128], idh_sb[:, :], idh_sb[:, :],
                    start=True, stop=True, skip_group_check=True,
                )
            tensor.wait_ge(csem["did"], 16)
            # rank-slice accumulation, 2 rank slices per DoubleRow matmul:
            # psum[p, :] += chunk[p, even, :] + chunk[p, odd, :]
            waited = -1
            for b in range(NBLK):
                for j in range(nb2[b]):
                    c = off[b] + j
                    gi = g_of_chunk[c]
                    if gi > waited:
                        tensor.wait_ge(xsem[gi], 16)
                        waited = gi
                    tensor.matmul(
                        ps[b][:, 0:H],
                        did_sb[:, :, :],
                        xbb[:, c, :].rearrange("p (n o) -> p o n", o=2),
                        start=(j == 0),
                        stop=(j == nb2[b] - 1),
                        skip_group_check=True,
                        perf_mode=mybir.MatmulPerfMode.DoubleRow,
                    )
                # fence: a matmul's then_inc can fire before its PSUM
                # writes drain; hand the bank to DVE only after a
                # trailing fence matmul completes
                tensor.matmul(
                    ps_x[:, 0:128], idh_sb[:, :], idh_sb[:, :],
                    start=True, stop=True, skip_group_check=True,
                ).then_inc(mmf_sem, 1)
            # transposes: pooled [s, h] -> pooled_T [h, s], per block
            for b in range(NBLK):
                tensor.wait_ge(fold_sem, b + 1)
                if b >= 2:
                    # ps_t[b%2] is read by DVE for block b-2; don't
                    # overwrite until that copy is done
                    tensor.wait_ge(cp2_sem, b - 1)
                for hb in range(2):
                    tensor.transpose(
                        ps_t[b % 2][:, 128 * hb : 128 * (hb + 1)],
                        pool_sb[:, b, 128 * hb : 128 * (hb + 1)],
                        idh_sb[:, :],
                    )
                tensor.matmul(
                    ps_x[:, 0:128], idh_sb[:, :], idh_sb[:, :],
                    start=True, stop=True, skip_group_check=True,
                ).then_inc(tr_sem, 1)
            # Linear: out[s, j] = sum_h pooled_T[h, s] * wt[h, j]
            tensor.wait_ge(csem["wt"], 16)
            for b in range(NBLK):
                tensor.wait_ge(cp2_sem, b + 1)
                tensor.matmul(
                    ps[b][:, 0:H],
                    sums2_sb[:, 0, 128 * b : 128 * (b + 1)],
                    wt_sb[:, 0, :],
                    start=True,
                    stop=False,
                    skip_group_check=True,
                )
                tensor.matmul(
                    ps[b][:, 0:H],
                    sums2_sb[:, 1, 128 * b : 128 * (b + 1)],
                    wt_sb[:, 1, :],
                    start=False,
                    stop=True,
                    skip_group_check=True,
                )
                tensor.matmul(
                    ps_x[:, 0:128], idh_sb[:, :], idh_sb[:, :],
                    start=True, stop=True, skip_group_check=True,
                ).then_inc(lin_sem, 1)

    return nc


def kernel(x, dst_idx, dst_size, W, b):
    x = np.asarray(x, dtype=np.float32)
    idx = np.asarray(dst_idx).astype(np.int64)
    W = np.asarray(W, dtype=np.float32)
    b = np.asarray(b, dtype=np.float32)
    S = int(dst_size)
    assert S == S_TOTAL and x.shape[1] == H

    counts = np.bincount(idx, minlength=S)
    inv = (np.float32(1.0) / (counts + EPS)).astype(np.float32)

    # deal count-sorted segments round-robin across cores; within a core
    # they stay count-sorted (ascending) -> blocks of 128 have near-equal
    # counts, so rank-slice padding is small
    seg_order = np.argsort(counts, kind="stable")  # [4096] ascending count
    seg_core = np.empty(S, dtype=np.int64)
    seg_pos = np.empty(S, dtype=np.int64)
    seg_core[seg_order] = np.arange(S) % N_CORES
    seg_pos[seg_order] = np.arange(S) // N_CORES

    # per-(core, block) max count -> shared schedule = max over cores, in
    # DoubleRow chunks of 2 rank slices each
    core_segs = [seg_order[c::N_CORES] for c in range(N_CORES)]  # sorted asc
    nb2 = []
    for blk in range(NBLK):
        m = max(int(counts[core_segs[c][128 * blk : 128 * (blk + 1)]].max())
                for c in range(N_CORES))
        nb2.append((m + 1) // 2)
    nb2 = tuple(nb2)
    ktot = sum(nb2)
    off = [sum(nb2[:blk]) for blk in range(NBLK)]

    nc = _graph_cache.get(nb2)
    if nc is None:
        nc = _build(nb2)
        _graph_cache[nb2] = nc

    # error-feedback fp8 quantization in segment-rank order: the running
    # residual of each (segment, feature) is carried into the next row,
    # so the segment sum telescopes to ~one element's rounding error
    order = np.argsort(idx, kind="stable")
    sidx = idx[order]
    starts = np.searchsorted(sidx, np.arange(S + 1))
    rank = np.arange(len(sidx)) - starts[sidx]
    xq = np.empty((len(idx), H), dtype=F8)
    err = np.zeros((S, H), dtype=np.float32)
    maxrank = int(rank.max())
    for r in range(maxrank + 1):
        sel = rank == r
        rows = order[sel]
        segs = sidx[sel]
        v = x[rows] + err[segs]
        q = v.astype(F8)
        err[segs] = v - q.astype(np.float32)
        xq[rows] = q

    # pack DoubleRow chunks: view [128, ktot, 256 features, 2 rank-parity]
    row_core = seg_core[sidx]
    row_pos = seg_pos[sidx]
    row_blk = row_pos // 128
    row_p = row_pos % 128
    row_chunk = np.asarray(off, dtype=np.int64)[row_blk] + rank // 2
    row_o = rank % 2

    did_np = np.zeros((128, 2, 128), dtype=F8)
    r128 = np.arange(128)
    did_np[r128, 0, r128] = 1.0
    did_np[r128, 1, r128] = 1.0
    idh_np = np.eye(128, dtype=np.float16)
    wt_np = np.ascontiguousarray(W.T).astype(np.float16)
    bb_np = np.ascontiguousarray(np.tile(b, (128, 1)), dtype=np.float32)

    in_maps = []
    for c in range(N_CORES):
        m = row_core == c
        xp = np.zeros((128, ktot, H, 2), dtype=F8)
        xp[row_p[m], row_chunk[m], :, row_o[m]] = xq[order[m]]
        invc_np = np.ascontiguousarray(
            inv[core_segs[c]].reshape(NBLK, 128).T
        )
        in_maps.append(
            {
                "xp": xp.reshape(128, ktot, 2 * H),
                "did": did_np,
                "idh": idh_np,
                "wt": wt_np,
                "invc": invc_np,
                "bb": bb_np,
            }
        )

    res = run_bass_kernel_spmd(nc, in_maps, core_ids=list(range(N_CORES)))
    out = np.empty((S, H), dtype=np.float32)
    for c in range(N_CORES):
        # device wrote [partition, block, H]; segment = core_segs[c][128b+p]
        o = res.results[c]["out"].astype(np.float32)
        out[core_segs[c]] = o.transpose(1, 0, 2).reshape(S_PER, H)
    return out


# revision 19
# speedup vs baseline: 1.0871x; 1.0871x over previous
"""Segment-mean pooling (segment_sum / counts) + Linear, on 8 TRN2 NeuronCores.

Strategy: segment-ownership sharding with rank-slice packing and fp8
DoubleRow matmuls.

The host sorts segments by count and deals them round-robin across the 8
cores (so per-core load is balanced).  Within a core its 512 segments are
kept count-sorted and split into 4 PSUM blocks of 128.  The host packs
the rows of x so that chunk c of block b holds, byte-interleaved in SBUF
partition p, rows 2c and 2c+1 of block b's p-th segment (zeros where the
segment has fewer rows).  Segment-summing a chunk is then ONE fp8
DoubleRow TensorE matmul (256 rows per ~109 ns) with a constant doubled
identity as the stationary operand: psum[p, :] += row2c[p, :] +
row2c1[p, :].  Full 128-wide matmuls keep the PE clock warm (HAM) and
need no per-row index handling, no one-hots, and no overflow pass.

x is shipped as fp8 e4m3 with per-segment error-feedback quantization:
rows of one segment are quantized in sequence, carrying the running
quantization residual into the next row, so the device-side segment sum
telescopes to full-precision accuracy minus ONE element's rounding
(final rel err ~5e-3 vs the 2e-2 gate) at half the f16 DMA bytes.  The
kernel is DMA-bound at ~350 GB/s; x streams in ~0.4 MB groups whose
completions are tracked with one cumulative semaphore (per-queue FIFO).

Epilogue per block: cast sums to f16, PE-transpose to [h, s], Linear via
2 accumulated matmuls against W.T, scale rows by 1/(count+eps) with
fused bias-add, one f16 output DMA (host upcasts and unpermutes).
"""

import numpy as np
import ml_dtypes

import concourse.bass as bass
import concourse.mybir as mybir
from concourse.bass_utils import run_bass_kernel_spmd

N_CORES = 8
S_TOTAL = 4096
S_PER = S_TOTAL // N_CORES  # 512 segments per core
NBLK = 4  # PSUM blocks of 128 segments per core
H = 256
EPS = np.float32(1e-8)
F8 = ml_dtypes.float8_e4m3  # matches mybir.dt.float8e4

N_WARM = 34  # PE warmup matmuls: one uninterrupted ~3.6 us run so the
# HAM clock transition to 2.4 GHz is guaranteed before data matmuls

_graph_cache: dict = {}


GCH = 5  # x DMA group size in chunks (5 * 64 KB = 320 KB)


def _groups(nb2):
    """x DMA groups of ~5 chunks: fine-grained arrival paces the PE's
    DoubleRow read bursts (which otherwise hog the shared SBUF fabric
    and stall the DMA engines' writes), while keeping the issue count
    low enough for the scalar ring.

    Returns a list of (start_chunk, n_chunks) in global chunk indexing.
    """
    total = sum(nb2)
    gs = []
    base = 0
    while base < total:
        n = min(GCH, total - base)
        gs.append((base, n))
        base += n
    return gs


def _build(nb2: tuple) -> "bass.Bass":
    """nb2[b] = DoubleRow chunk count of block b (same on every core)."""
    f8 = mybir.dt.float8e4
    f16 = mybir.dt.float16
    f32 = mybir.dt.float32
    ktot = sum(nb2)
    off = [sum(nb2[:b]) for b in range(NBLK)]
    groups = _groups(nb2)
    # group index that completes chunk c, for wait thresholds
    g_of_chunk = {}
    for gi, (c0, n) in enumerate(groups):
        for c in range(c0, c0 + n):
            g_of_chunk[c] = gi

    nc = bass.Bass()

    xp_d = nc.declare_dram_parameter("xp", [128, ktot, 2 * H], f8, isOutput=False)
    did_d = nc.declare_dram_parameter("did", [128, 2, 128], f8, isOutput=False)
    idh_d = nc.declare_dram_parameter("idh", [128, 128], f16, isOutput=False)
    wt_d = nc.declare_dram_parameter("wt", [H, H], f16, isOutput=False)
    invc_d = nc.declare_dram_parameter("invc", [128, NBLK], f32, isOutput=False)
    bb_d = nc.declare_dram_parameter("bb", [128, H], f32, isOutput=False)
    # [partition, block, H]: contiguous DMA; the host un-interleaves
    out_d = nc.declare_dram_parameter("out", [128, NBLK, H], f16, isOutput=True)

    from contextlib import ExitStack

    with ExitStack() as ctx:
        xbb = ctx.enter_context(nc.sbuf_tensor("xbb", [128, ktot, 2 * H], f8))
        did_sb = ctx.enter_context(nc.sbuf_tensor("did_sb", [128, 2, 128], f8))
        idh_sb = ctx.enter_context(nc.sbuf_tensor("idh_sb", [128, 128], f16))
        wt_sb = ctx.enter_context(nc.sbuf_tensor("wt_sb", [128, 2, H], f16))
        invc_sb = ctx.enter_context(nc.sbuf_tensor("invc_sb", [128, NBLK], f32))
        bb_sb = ctx.enter_context(nc.sbuf_tensor("bb_sb", [128, H], f32))
        pool_sb = ctx.enter_context(nc.sbuf_tensor("pool_sb", [128, NBLK, H], f16))
        sums2_sb = ctx.enter_context(
            nc.sbuf_tensor("sums2_sb", [128, 2, S_PER], f16)
        )
        out_sb = ctx.enter_context(nc.sbuf_tensor("out_sb", [128, NBLK, H], f16))
        # every PSUM tensor padded to one full private 2 KiB bank
        ps = [
            ctx.enter_context(nc.psum_tensor(f"ps{b}", [128, 512], f32))
            for b in range(NBLK)
        ]
        ps_t = [
            ctx.enter_context(nc.psum_tensor(f"ps_t{i}", [128, 1024], f16))
            for i in range(2)
        ]
        ps_x = ctx.enter_context(nc.psum_tensor("ps_x", [128, 512], f32))
        dma_sem = ctx.enter_context(nc.semaphore("dma_sem"))
        csem = {
            name: ctx.enter_context(nc.semaphore(f"csem_{name}"))
            for name in ("did", "idh", "wt", "invc", "bb")
        }
        xsem = [
            ctx.enter_context(nc.semaphore(f"xsem{g}"))
            for g in range(len(groups))
        ]
        mmf_sem = ctx.enter_context(nc.semaphore("mmf_sem"))
        fold_sem = ctx.enter_context(nc.semaphore("fold_sem"))
        tr_sem = ctx.enter_context(nc.semaphore("tr_sem"))
        cp2_sem = ctx.enter_context(nc.semaphore("cp2_sem"))
        lin_sem = ctx.enter_context(nc.semaphore("lin_sem"))
        oe_sem = ctx.enter_context(nc.semaphore("oe_sem"))
        block = ctx.enter_context(nc.Block())

        @block.sync
        def _(sync):
            # late-needed consts only: this queue can starve for a few us
            # behind the x stream, which is fine here
            sync.dma_start(
                out=wt_sb[:, :, :],
                in_=wt_d[:, :].rearrange("(t p) j -> p t j", p=128),
            ).then_inc(csem["wt"], 16)
            sync.dma_start(out=invc_sb[:, :], in_=invc_d[:, :]).then_inc(
                csem["invc"], 16
            )
            sync.dma_start(out=bb_sb[:, :], in_=bb_d[:, :]).then_inc(
                csem["bb"], 16
            )
            sync.wait_ge(oe_sem, NBLK)
            sync.dma_start(
                out=out_d[:, :, :], in_=out_sb[:, :, :]
            ).then_inc(dma_sem, 16)
            for name in ("wt", "invc", "bb"):
                sync.wait_ge(csem[name], 16)
            sync.wait_ge(dma_sem, 16)

        @block.scalar
        def _(scalar):
            # time-critical consts FIRST on the same queue as x (FIFO):
            # idh gates the PE warmup, did gates the data matmuls
            scalar.dma_start(out=idh_sb[:, :], in_=idh_d[:, :]).then_inc(
                csem["idh"], 16
            )
            scalar.dma_start(out=did_sb[:, :, :], in_=did_d[:, :, :]).then_inc(
                csem["did"], 16
            )
            # one semaphore per group: groups can land out of order (DMAs
            # round-robin across queues), so a shared cumulative
            # threshold can't tell WHICH group completed
            for gi, (c0, n) in enumerate(groups):
                scalar.dma_start(
                    out=xbb[:, c0 : c0 + n, :],
                    in_=xp_d[:, c0 : c0 + n, :],
                ).then_inc(xsem[gi], 16)
            scalar.wait_ge(csem["idh"], 16)
            scalar.wait_ge(csem["did"], 16)
            for gi in range(len(groups)):
                scalar.wait_ge(xsem[gi], 16)

        @block.vector
        def _(vector):
            def fold(b):
                vector.wait_ge(mmf_sem, b + 1)
                vector.tensor_copy(
                    out=pool_sb[:, b, :], in_=ps[b][:, 0:H]
                ).then_inc(fold_sem, 1)

            def cp2(b):
                vector.wait_ge(tr_sem, b + 1)
                vector.tensor_copy(
                    out=sums2_sb[:, 0, 128 * b : 128 * (b + 1)],
                    in_=ps_t[b % 2][:, 0:128],
                )
                vector.tensor_copy(
                    out=sums2_sb[:, 1, 128 * b : 128 * (b + 1)],
                    in_=ps_t[b % 2][:, 128:256],
                ).then_inc(cp2_sem, 1)

            # interleaved to match the PE schedule (no circular waits):
            # PE: D0 D1 T0 D2 T1 L0 D3 T2 L1 T3 L2 L3
            fold(0)
            fold(1)
            cp2(0)
            fold(2)
            cp2(1)
            fold(3)
            cp2(2)
            cp2(3)
            vector.wait_ge(csem["invc"], 16)
            vector.wait_ge(csem["bb"], 16)
            for b in range(NBLK):
                vector.wait_ge(lin_sem, b + 1)
                vector.scalar_tensor_tensor(
                    out=out_sb[:, b, :],
                    in0=ps[b][:, 0:H],
                    scalar=invc_sb[:, b : b + 1],
                    in1=bb_sb[:, :],
                    op0=mybir.AluOpType.mult,
                    op1=mybir.AluOpType.add,
                ).then_inc(oe_sem, 1)

        @block.tensor
        def _(tensor):
            # HAM warmup: sustained activity from the earliest moment so
            # the PE is at 2.4 GHz when the first data group lands
            tensor.wait_ge(csem["idh"], 16)
            for _ in range(N_WARM):
                tensor.matmul(
                    ps_x[:, 0:128], idh_sb[:, :], idh_sb[:, :],
                    start=True, stop=True, skip_group_check=True,
                )
            tensor.wait_ge(csem["did"], 16)

            def fence(sem):
                tensor.matmul(
                    ps_x[:, 0:128], idh_sb[:, :], idh_sb[:, :],
                    start=True, stop=True, skip_group_check=True,
                ).then_inc(sem, 1)

            waited = [-1]

            def data(b):
                # rank-slice accumulation, 2 rank slices per DoubleRow
                # matmul: psum[p, :] += chunk[p, even, :] + chunk[p, odd, :]
                for j in range(nb2[b]):
                    c = off[b] + j
                    gi = g_of_chunk[c]
                    if gi > waited[0]:
                        tensor.wait_ge(xsem[gi], 16)
                        waited[0] = gi
                    tensor.matmul(
                        ps[b][:, 0:H],
                        did_sb[:, :, :],
                        xbb[:, c, :].rearrange("p (n o) -> p o n", o=2),
                        start=(j == 0),
                        stop=(j == nb2[b] - 1),
                        skip_group_check=True,
                        perf_mode=mybir.MatmulPerfMode.DoubleRow,
                    )
                # fence: a matmul's then_inc can fire before its PSUM
                # writes drain; hand the bank to DVE only after a
                # trailing fence matmul completes
                fence(mmf_sem)

            def transp(b):
                # pooled [s, h] -> pooled_T [h, s]
                tensor.wait_ge(fold_sem, b + 1)
                if b >= 2:
                    # ps_t[b%2] is read by DVE for block b-2; don't
                    # overwrite until that copy is done
                    tensor.wait_ge(cp2_sem, b - 1)
                for hb in range(2):
                    tensor.transpose(
                        ps_t[b % 2][:, 128 * hb : 128 * (hb + 1)],
                        pool_sb[:, b, 128 * hb : 128 * (hb + 1)],
                        idh_sb[:, :],
                    )
                fence(tr_sem)

            def linear(b):
                # out[s, j] = sum_h pooled_T[h, s] * wt[h, j]
                if b == 0:
                    tensor.wait_ge(csem["wt"], 16)
                tensor.wait_ge(cp2_sem, b + 1)
                tensor.matmul(
                    ps[b][:, 0:H],
                    sums2_sb[:, 0, 128 * b : 128 * (b + 1)],
                    wt_sb[:, 0, :],
                    start=True,
                    stop=False,
                    skip_group_check=True,
                )
                tensor.matmul(
                    ps[b][:, 0:H],
                    sums2_sb[:, 1, 128 * b : 128 * (b + 1)],
                    wt_sb[:, 1, :],
                    start=False,
                    stop=True,
                    skip_group_check=True,
                )
                fence(lin_sem)

            # epilogue stages interleaved between data blocks: they fill
            # PE idle slots while waiting on the DMA stream, leaving only
            # block 3's chain in the tail
            data(0)
            data(1)
            transp(0)
            data(2)
            transp(1)
            linear(0)
            data(3)
            transp(2)
            linear(1)
            transp(3)
            linear(2)
            linear(3)

    return nc


def kernel(x, dst_idx, dst_size, W, b):
    x = np.asarray(x, dtype=np.float32)
    idx = np.asarray(dst_idx).astype(np.int64)
    W = np.asarray(W, dtype=np.float32)
    b = np.asarray(b, dtype=np.float32)
    S = int(dst_size)
    assert S == S_TOTAL and x.shape[1] == H

    counts = np.bincount(idx, minlength=S)
    inv = (np.float32(1.0) / (counts + EPS)).astype(np.float32)

    # deal count-sorted segments round-robin across cores; within a core
    # they stay count-sorted (ascending) -> blocks of 128 have near-equal
    # counts, so rank-slice padding is small
    seg_order = np.argsort(counts, kind="stable")  # [4096] ascending count
    seg_core = np.empty(S, dtype=np.int64)
    seg_pos = np.empty(S, dtype=np.int64)
    seg_core[seg_order] = np.arange(S) % N_CORES
    seg_pos[seg_order] = np.arange(S) // N_CORES

    # per-(core, block) max count -> shared schedule = max over cores, in
    # DoubleRow chunks of 2 rank slices each
    core_segs = [seg_order[c::N_CORES] for c in range(N_CORES)]  # sorted asc
    nb2 = []
    for blk in range(NBLK):
        m = max(int(counts[core_segs[c][128 * blk : 128 * (blk + 1)]].max())
                for c in range(N_CORES))
        nb2.append((m + 1) // 2)
    nb2 = tuple(nb2)
    ktot = sum(nb2)
    off = [sum(nb2[:blk]) for blk in range(NBLK)]

    nc = _graph_cache.get(nb2)
    if nc is None:
        nc = _build(nb2)
        _graph_cache[nb2] = nc

    # error-feedback fp8 quantization in segment-rank order: the running
    # residual of each (segment, feature) is carried into the next row,
    # so the segment sum telescopes to ~one element's rounding error
    order = np.argsort(idx, kind="stable")
    sidx = idx[order]
    starts = np.searchsorted(sidx, np.arange(S + 1))
    rank = np.arange(len(sidx)) - starts[sidx]
    xq = np.empty((len(idx), H), dtype=F8)
    err = np.zeros((S, H), dtype=np.float32)
    maxrank = int(rank.max())
    for r in range(maxrank + 1):
        sel = rank == r
        rows = order[sel]
        segs = sidx[sel]
        v = x[rows] + err[segs]
        q = v.astype(F8)
        err[segs] = v - q.astype(np.float32)
        xq[rows] = q

    # pack DoubleRow chunks: view [128, ktot, 256 features, 2 rank-parity]
    row_core = seg_core[sidx]
    row_pos = seg_pos[sidx]
    row_blk = row_pos // 128
    row_p = row_pos % 128
    row_chunk = np.asarray(off, dtype=np.int64)[row_blk] + rank // 2
    row_o = rank % 2

    did_np = np.zeros((128, 2, 128), dtype=F8)
    r128 = np.arange(128)
    did_np[r128, 0, r128] = 1.0
    did_np[r128, 1, r128] = 1.0
    idh_np = np.eye(128, dtype=np.float16)
    wt_np = np.ascontiguousarray(W.T).astype(np.float16)
    bb_np = np.ascontiguousarray(np.tile(b, (128, 1)), dtype=np.float32)

    in_maps = []
    for c in range(N_CORES):
        m = row_core == c
        xp = np.zeros((128, ktot, H, 2), dtype=F8)
        xp[row_p[m], row_chunk[m], :, row_o[m]] = xq[order[m]]
        invc_np = np.ascontiguousarray(
            inv[core_segs[c]].reshape(NBLK, 128).T
        )
        in_maps.append(
            {
                "xp": xp.reshape(128, ktot, 2 * H),
                "did": did_np,
                "idh": idh_np,
                "wt": wt_np,
                "invc": invc_np,
                "bb": bb_np,
            }
        )

    res = run_bass_kernel_spmd(nc, in_maps, core_ids=list(range(N_CORES)))
    out = np.empty((S, H), dtype=np.float32)
    for c in range(N_CORES):
        # device wrote [partition, block, H]; segment = core_segs[c][128b+p]
        o = res.results[c]["out"].astype(np.float32)
        out[core_segs[c]] = o.transpose(1, 0, 2).reshape(S_PER, H)
    return out
